# revision 23
# baseline (speedup 1.0000x reference)
"""Trainium2 Bass kernel for a custom attention block (qkv-proj + LN(q,k) +
RoPE + causal attention + out-proj), distributed over 8 NeuronCores.

Sharding: 2 cores per batch (B=4). Core role r=c%2 takes q-token blocks
{0,3} (r=0) or {1,2} (r=1) of 512 tokens; every core computes K/V for the
full 2048-token sequence of its batch (no collectives). The compiled
program is identical on all cores; per-core differences are input data.

v2 design (vs the fp32r baseline):
 - All matmul operands are bf16 (psum accumulation stays fp32): 1 PE
   cycle/row at every pstate, half the DMA/SBUF bytes.
 - Every weight byte is read from HBM exactly once per core: projections
   are weight-stationary (ec-loop outside the token-slice loop) with x
   SBUF-resident; out-proj streams each wo block once.
 - q stays in SBUF through attention; k/v round-trip DRAM in bf16.
 - Attention is software-pipelined: the scores matmul for slot s+2 is
   emitted before PV/den of slot s, so the in-order PE queue never
   head-of-line blocks on the scalar-engine exp; each head's epilogue
   (reciprocal/broadcast/normalize) is deferred behind the next head's
   first two score matmuls.
 - The PE pstate ramp (2.4 GHz only after ~3us of continuous execution)
   is why the matmul streams are kept as long and gap-free as possible.
"""

import math

import numpy as np

import concourse.bass as bass
import concourse.mybir as mybir
import concourse.tile as tile
from concourse import bacc
from concourse import bass_isa
from concourse.bass import ds

F32 = mybir.dt.float32
F32R = mybir.dt.float32r
BF16 = mybir.dt.bfloat16
AF = mybir.ActivationFunctionType
OP = mybir.AluOpType


def _r(ap):
    """fp32 -> fp32r view for matmul operands."""
    return ap.bitcast(F32R)

P = 128
HD = 128

FULL_CFG = dict(
    D=2048,           # model dim
    S=2048,           # kv tokens per core
    NQTOK=1024,       # q tokens per core
    QT=512,           # token-slice / attention q-tile width
    slots=(8, 16),    # kv 128-chunks visited per q-tile
    masked=(tuple(range(0, 8)), tuple(range(8, 16))),
    EXP_BIAS=8.0,
    EPS=1e-5,
)


def build_program(cfg):
    D = cfg["D"]
    S = cfg["S"]
    NQTOK = cfg["NQTOK"]
    QT = cfg["QT"]
    slots = cfg["slots"]
    masked = cfg["masked"]
    EXP_BIAS = cfg["EXP_BIAS"]
    EPS = cfg["EPS"]

    NH = D // HD              # heads == feature chunks
    DC = D // P               # contraction chunks
    NQ = NQTOK // QT          # q tiles
    NKC = S // P              # kv chunks
    QSL = NQTOK // QT         # q slices
    KSL = S // QT             # k slices
    MAXM = max(len(m) for m in masked)
    H2 = HD // 2

    nc = bacc.Bacc("TRN2", target_bir_lowering=False, debug=False)

    # ---- I/O (bf16 unless noted) ----
    xT_i = nc.dram_tensor("xT", [DC, P, S], BF16, kind="ExternalInput").ap()
    xq_i = nc.dram_tensor("xq", [P, DC, NQTOK], BF16,
                          kind="ExternalInput").ap()
    wqk_i = nc.dram_tensor("wqk", [2 * NH, P, DC, P], BF16,
                           kind="ExternalInput").ap()
    wv_i = nc.dram_tensor("wv", [P, DC, D], BF16, kind="ExternalInput").ap()
    wo_i = nc.dram_tensor("wo", [NH, P, D], BF16, kind="ExternalInput").ap()
    qcos_i = nc.dram_tensor("qcos", [P, NQTOK], BF16, kind="ExternalInput").ap()
    qsin_i = nc.dram_tensor("qsin", [P, NQTOK], BF16, kind="ExternalInput").ap()
    kcos_i = nc.dram_tensor("kcos", [P, S], BF16, kind="ExternalInput").ap()
    ksin_i = nc.dram_tensor("ksin", [P, S], BF16, kind="ExternalInput").ap()
    onesc_i = nc.dram_tensor("onesc", [P, 1], BF16, kind="ExternalInput").ap()
    onesr_i = nc.dram_tensor("onesr", [1, P], BF16, kind="ExternalInput").ap()
    onescf_i = nc.dram_tensor("onescf", [P, 1], F32, kind="ExternalInput").ap()
    onesrf_i = nc.dram_tensor("onesrf", [1, P], F32, kind="ExternalInput").ap()
    gq_i = nc.dram_tensor("gq", [P, NH], F32, kind="ExternalInput").ap()
    bq_i = nc.dram_tensor("bq", [P, NH], F32, kind="ExternalInput").ap()
    gk_i = nc.dram_tensor("gk", [P, NH], F32, kind="ExternalInput").ap()
    bk_i = nc.dram_tensor("bk", [P, NH], F32, kind="ExternalInput").ap()
    masks_i = nc.dram_tensor("masks", [NQ, P, MAXM, QT], BF16,
                             kind="ExternalInput").ap()
    out_t = nc.dram_tensor("out", [D, NQTOK], F32, kind="ExternalOutput").ap()

    with tile.TileContext(nc) as tc:
        import contextlib

        ctx = contextlib.ExitStack()
        with ctx:
            sb = ctx.enter_context(tc.tile_pool(name="sb", bufs=1))
            psum = ctx.enter_context(tc.tile_pool(name="ps", bufs=1,
                                                  space="PSUM"))
            dram = ctx.enter_context(tc.tile_pool(name="dram", bufs=1,
                                                  space="DRAM"))

            # ---- DRAM scratch ----
            kts = dram.tile([P, NH, S], BF16, tag="kts", name="kts")
            vs2 = dram.tile([NH, P, NKC, HD], BF16, tag="vs2", name="vs2")

            # ---- constants / small inputs ----
            ones_row = sb.tile([1, P], BF16, tag="onesr", name="ones_row")
            nc.sync.dma_start(ones_row, onesr_i)
            ones_col_f = sb.tile([P, 1], F32, tag="onescf", name="ones_col_f")
            nc.sync.dma_start(_r(ones_col_f), _r(onescf_i))
            ones_row_f = sb.tile([1, P], F32, tag="onesrf", name="ones_row_f")
            nc.sync.dma_start(_r(ones_row_f), _r(onesrf_i))
            eps1 = sb.tile([1, 1], F32, tag="eps1", name="eps1")
            nc.vector.memset(eps1, EPS)
            zero1 = sb.tile([1, 1], F32, tag="zero1", name="zero1")
            nc.vector.memset(zero1, 0.0)
            nege = sb.tile([P, 1], F32, tag="nege", name="nege")
            nc.vector.memset(nege, -EXP_BIAS)
            gq = sb.tile([P, NH], F32, tag="gq", name="gq")
            nc.sync.dma_start(gq, gq_i)
            bq = sb.tile([P, NH], F32, tag="bq", name="bq")
            nc.sync.dma_start(bq, bq_i)
            gk = sb.tile([P, NH], F32, tag="gk", name="gk")
            nc.sync.dma_start(gk, gk_i)
            bk = sb.tile([P, NH], F32, tag="bk", name="bk")
            nc.sync.dma_start(bk, bk_i)

            # x resident; DMAs emitted after the q-pass inputs
            xdc = [
                sb.tile([P, S], BF16, tag="xdc", bufs=DC, name=f"x_{d}")
                for d in range(DC)
            ]

            # q result resident through attention
            qres = sb.tile([P, NH, NQTOK], BF16, tag="qres", name="qres")

            def ropetab(src, cols, nm):
                t = sb.tile([P, QT], BF16, tag="ropetab", bufs=4, name=nm)
                nc.sync.dma_start(t, src[:, cols])
                return t

            # ---------------------------------------------------------
            # Projection building blocks
            # ---------------------------------------------------------
            def qk_mm_pass(w_off, rhs_of, nsl, hold_of, extra, drain):
                """Weight-stationary q/k projection pass; drains `drain`
                deferred DVE thunks from `extra` after each (ec, sl)
                matmul group so the DVE stream advances with the PE."""
                sqaccs = [
                    sb.tile([P, QT], F32, tag="sqacc", bufs=2, name="sqacc")
                    for _ in range(nsl)
                ]
                for ec in range(NH):
                    w = sb.tile([P, DC, P], BF16, tag="wsl", bufs=2,
                                name="wsl")
                    nc.sync.dma_start(w, wqk_i[w_off + ec])
                    for sl in range(nsl):
                        ps = psum.tile([P, QT], F32, tag="mm", bufs=3,
                                       name="ps_qk")
                        for d in range(DC):
                            nc.tensor.matmul(
                                ps,
                                lhsT=w[:, d],
                                rhs=rhs_of(sl, d),
                                start=(d == 0),
                                stop=(d == DC - 1),
                            )
                        hold = hold_of(sl, ec)
                        nc.vector.tensor_copy(hold, ps)
                        if ec == 0:
                            nc.vector.tensor_tensor(
                                _r(sqaccs[sl]), hold, hold, op=OP.mult
                            )
                        else:
                            sq = sb.tile([P, QT], BF16, tag="sq", bufs=2,
                                         name="sq")
                            nc.vector.tensor_tensor(sq, hold, hold,
                                                    op=OP.mult)
                            nc.vector.tensor_tensor(
                                _r(sqaccs[sl]), sqaccs[sl], sq, op=OP.add
                            )
                        for _ in range(drain):
                            if extra:
                                extra.pop(0)()
                return sqaccs

            def qk_stats(sqaccs):
                """Per-slice rsig broadcast psums (PE/ACT only)."""
                reps = []
                for sa in sqaccs:
                    pstat = psum.tile([1, QT], F32, tag="stat", bufs=3,
                                      name="pstat")
                    nc.tensor.matmul(pstat, lhsT=_r(ones_col_f), rhs=_r(sa))
                    lnv = sb.tile([1, QT], F32, tag="stats_sb", bufs=2,
                                  name="lnv")
                    nc.scalar.activation(lnv, pstat, AF.Ln, scale=1.0 / D,
                                         bias=eps1)
                    rsig = sb.tile([1, QT], F32, tag="stats_sb", bufs=2,
                                   name="rsig")
                    nc.scalar.activation(_r(rsig), lnv, AF.Exp, bias=zero1,
                                         scale=-0.5)
                    rep = psum.tile([P, QT], F32, tag="stat", bufs=3,
                                    name="ps_rep")
                    nc.tensor.matmul(rep, lhsT=_r(ones_row_f), rhs=_r(rsig))
                    reps.append(rep)
                return reps

            def ln_rope_thunks(reps, hold_of, g_sb, b_sb, cos_t, sin_t,
                               col_of, post):
                """Deferred LN+RoPE work as a thunk list: per (sl, ec) a
                part-A (LN apply, w = ch*(-B), swap-DMA, ch *= A) and a
                part-B (ch += t2) staggered 2 groups later so the swap DMA
                latency never stalls the DVE queue. `post[sl]` (e.g. the
                kts writeback) is appended after slice sl's last part-B."""
                out = []
                for sl, rep in enumerate(reps):
                    col = col_of(sl)
                    pend = []

                    def mk_a(sl=sl, rep=rep, col=col, pend=pend):
                        def part_a(ec):
                            def f():
                                ch = hold_of(sl, ec)
                                nc.vector.tensor_tensor(ch, ch, rep,
                                                        op=OP.mult)
                                nc.vector.tensor_scalar(
                                    ch, ch,
                                    scalar1=g_sb[:, ds(ec, 1)],
                                    scalar2=b_sb[:, ds(ec, 1)],
                                    op0=OP.mult, op1=OP.add,
                                )
                                w = sb.tile([P, QT], BF16, tag="tmp",
                                            bufs=3, name="rw")
                                nc.vector.tensor_tensor(
                                    w, ch, sin_t[:, col], op=OP.mult
                                )
                                t2 = sb.tile([P, QT], BF16, tag="rswp",
                                             bufs=3, name="t2")
                                nc.sync.dma_start(t2[ds(0, H2)],
                                                  w[ds(H2, H2)])
                                nc.sync.dma_start(t2[ds(H2, H2)],
                                                  w[ds(0, H2)])
                                nc.vector.tensor_tensor(
                                    ch, ch, cos_t[:, col], op=OP.mult
                                )
                                pend.append((ec, t2))
                            return f

                        def part_b():
                            ec, t2 = pend.pop(0)
                            ch = hold_of(sl, ec)
                            nc.vector.tensor_tensor(ch, ch, t2, op=OP.add)
                        return part_a, part_b

                    part_a, part_b = mk_a()
                    for ec in range(NH):
                        out.append(part_a(ec))
                        if ec >= 2:
                            out.append(part_b)
                    out.append(part_b)
                    out.append(part_b)
                    if post[sl] is not None:
                        out.append(post[sl])
                return out

            # ================= Projection phase ======================
            # ---- q: mm pass (x-q slices via slab tag b0,b1) ----
            xqsl = []
            for sl in range(QSL):
                t = sb.tile([P, DC, QT], BF16, tag="slab", bufs=3,
                            name="xqsl")
                nc.sync.dma_start(t, xq_i[:, :, ds(sl * QT, QT)])
                xqsl.append(t)

            def q_hold(sl, ec):
                return qres[:, ec, ds(sl * QT, QT)]

            q_sqaccs = qk_mm_pass(
                0, lambda sl, d: xqsl[sl][:, d], QSL, q_hold, [], 0
            )
            for d in range(DC):
                nc.sync.dma_start(xdc[d], xT_i[d])
            q_reps = qk_stats(q_sqaccs)
            qtabc = [ropetab(qcos_i, ds(sl * QT, QT), "qcos_t")
                     for sl in range(QSL)]
            qtabs = [ropetab(qsin_i, ds(sl * QT, QT), "qsin_t")
                     for sl in range(QSL)]
            pending = []
            for sl in range(QSL):
                pending += ln_rope_thunks(
                    [q_reps[sl]],
                    lambda s, ec, sl=sl: q_hold(sl, ec),
                    gq, bq, qtabc[sl], qtabs[sl],
                    lambda s: ds(0, QT), [None],
                )

            # ---- k: 4 single-slice passes, each draining the previous
            # pass's LN/rope thunks into its matmul stream ----
            k_slabs = []
            for ksl_i in range(KSL):
                slab = sb.tile([P, NH, QT], BF16, tag="slab", bufs=3,
                               name=f"khold{ksl_i}")
                k_slabs.append(slab)

                def k_hold(sl, ec, slab=slab):
                    return slab[:, ec]

                k_sqaccs = qk_mm_pass(
                    NH,
                    lambda sl, d, ksl_i=ksl_i: xdc[d][:, ds(ksl_i * QT, QT)],
                    1, k_hold, pending, 4,
                )
                while pending:
                    pending.pop(0)()
                k_reps = qk_stats(k_sqaccs)
                ktc = ropetab(kcos_i, ds(ksl_i * QT, QT), f"kcos{ksl_i}")
                kts_ = ropetab(ksin_i, ds(ksl_i * QT, QT), f"ksin{ksl_i}")

                def mk_post(ksl_i=ksl_i, slab=slab):
                    def f():
                        nc.sync.dma_start(
                            kts[:, :, ds(ksl_i * QT, QT)], slab
                        )
                    return f

                pending = ln_rope_thunks(
                    k_reps, k_hold, gk, bk, ktc, kts_,
                    lambda s: ds(0, QT), [mk_post()],
                )

            # ---- v projection (eg 0 drains the last k slice's thunks) --
            def v_eg(eg, extra, drain):
                wv_eg = sb.tile([P, DC, 512], BF16, tag="slab", bufs=3,
                                name="wv_eg")
                nc.sync.dma_start(wv_eg, wv_i[:, :, ds(eg * 512, 512)])
                for kc in range(NKC):
                    ps = psum.tile([P, 512], F32, tag="mm", bufs=3,
                                   name="ps_v")
                    for d in range(DC):
                        nc.tensor.matmul(
                            ps,
                            lhsT=xdc[d][:, ds(kc * P, P)],
                            rhs=wv_eg[:, d],
                            start=(d == 0),
                            stop=(d == DC - 1),
                        )
                    vsb = sb.tile([P, 512], BF16, tag="vsb", bufs=2,
                                  name="vsb")
                    nc.vector.tensor_copy(vsb, ps)
                    for hh in range(4):
                        nc.sync.dma_start(
                            vs2[eg * 4 + hh][:, kc, :],
                            vsb[:, ds(hh * HD, HD)],
                        )
                    for _ in range(drain):
                        if extra:
                            extra.pop(0)()

            v_eg(0, pending, 3)
            while pending:
                pending.pop(0)()
            for eg in range(1, D // 512):
                v_eg(eg, [], 0)

            # ================= Attention phase =======================
            ot_slabs = {}
            pend_rec = []   # (psout, rec, t, h) after den: rec computed
            pend_ot = []    # (psout, rsb, t, h) after psr: ot pending

            def tail_rec(psout, psden, t, h):
                rec0 = sb.tile([1, QT], F32, tag="stats_sb", bufs=2,
                               name="rec0")
                with nc.allow_low_precision(
                    reason="softmax denominator reciprocal"
                ):
                    nc.vector.reciprocal_approx_fast(rec0, psden)
                rec = sb.tile([1, QT], BF16, tag="stats_sb", bufs=2,
                              name="rec")
                nc.scalar.activation(rec, rec0, AF.Copy)
                pend_rec.append((psout, rec, t, h))

            def tail_psr():
                psout, rec, t, h = pend_rec.pop(0)
                psr = psum.tile([P, QT], F32, tag="stat", bufs=3,
                                name="psr")
                nc.tensor.matmul(psr, lhsT=ones_row, rhs=rec)
                rsb = sb.tile([P, QT], BF16, tag="tmp", bufs=3, name="rsb")
                nc.scalar.activation(rsb, psr, AF.Copy)
                pend_ot.append((psout, rsb, t, h))

            def tail_ot():
                psout, rsb, t, h = pend_ot.pop(0)
                nc.vector.tensor_tensor(
                    ot_slabs[t][:, h], psout, rsb, op=OP.mult
                )

            for t in range(NQ):
                n_slots = slots[t]
                mpos = {kc: i for i, kc in enumerate(masked[t])}
                ot_slabs[t] = sb.tile([P, NH, QT], BF16, tag="slab",
                                      bufs=3, name=f"ot_slab{t}")
                mt = sb.tile([P, MAXM, QT], BF16, tag="masks", bufs=1,
                             name="mt")
                nc.sync.dma_start(mt, masks_i[t])
                n_half = (n_slots + 7) // 8
                for h in range(NH):
                    qv = qres[:, h, ds(t * QT, QT)]
                    kslh = []
                    vslh = []
                    for hf in range(n_half):
                        kt = sb.tile([P, 8 * P], BF16, tag="kslab",
                                     bufs=3, name="kslh")
                        nc.sync.dma_start(
                            kt, kts[:, h, ds(hf * 8 * P, 8 * P)]
                        )
                        kslh.append(kt)
                        vt = sb.tile([P, 8, HD], BF16, tag="vslab",
                                     bufs=3, name="vslh")
                        nc.sync.dma_start(
                            vt, vs2[h][:, ds(hf * 8, 8), :]
                        )
                        vslh.append(vt)
                    psout = psum.tile([P, QT], F32, tag="acc", bufs=2,
                                      name="psout")
                    den_acc = sb.tile([1, QT], F32, tag="denacc", bufs=1,
                                      name="den_acc")

                    ets = {}

                    def emit_score(s):
                        pss = psum.tile([P, QT], F32, tag="mm", bufs=3,
                                        name="pss")
                        nc.tensor.matmul(
                            pss,
                            lhsT=kslh[s // 8][:, ds((s % 8) * P, P)],
                            rhs=qv,
                        )
                        et = sb.tile([P, QT], BF16, tag="et", bufs=3,
                                     name="et")
                        nc.scalar.activation(et, pss, AF.Exp, bias=nege)
                        if s in mpos:
                            nc.vector.tensor_tensor(
                                et, et, mt[:, mpos[s]], op=OP.mult
                            )
                        ets[s] = et

                    def emit_pv(s):
                        nc.tensor.matmul(
                            psout,
                            lhsT=vslh[s // 8][:, s % 8, :],
                            rhs=ets[s],
                            start=(s == 0),
                            stop=(s == n_slots - 1),
                        )
                        # denominator on the (otherwise idle) gpsimd
                        # engine: per-slot partition all-reduce, then a
                        # tiny [1, QT] accumulate on DVE
                        denrep = sb.tile([P, QT], BF16, tag="denrep",
                                         bufs=2, name="denrep")
                        nc.gpsimd.partition_all_reduce(
                            denrep, ets[s], channels=P,
                            reduce_op=bass_isa.ReduceOp.add,
                        )
                        if s == 0:
                            nc.vector.tensor_copy(den_acc,
                                                  denrep[ds(0, 1)])
                        else:
                            nc.vector.tensor_tensor(
                                den_acc, den_acc, denrep[ds(0, 1)],
                                op=OP.add,
                            )
                        del ets[s]

                    emit_score(0)
                    emit_score(1)
                    if pend_rec:
                        tail_psr()
                    for s in range(n_slots):
                        if s + 2 < n_slots:
                            emit_score(s + 2)
                        emit_pv(s)
                        if s == 0 and pend_ot:
                            tail_ot()
                    tail_rec(psout, den_acc, t, h)
            tail_psr()
            tail_ot()

            # ================= Out-projection ========================
            for g in range(NH // 2):
                psf = [
                    psum.tile([P, QT], F32, tag="mm", bufs=3, name="psf"),
                    psum.tile([P, QT], F32, tag="mm", bufs=3, name="psf"),
                    psum.tile([P, QT], F32, tag="acc", bufs=2, name="psf"),
                    psum.tile([P, QT], F32, tag="acc", bufs=2, name="psf"),
                ]
                for h in range(NH):
                    wot = sb.tile([P, 2 * P], BF16, tag="wot", bufs=3,
                                  name="wot")
                    nc.sync.dma_start(
                        wot, wo_i[h][:, ds(g * 2 * P, 2 * P)]
                    )
                    for i in range(2):
                        for t in range(NQ):
                            nc.tensor.matmul(
                                psf[i * 2 + t],
                                lhsT=wot[:, ds(i * P, P)],
                                rhs=ot_slabs[t][:, h],
                                start=(h == 0),
                                stop=(h == NH - 1),
                            )
                for i in range(2):
                    for t in range(NQ):
                        fsb = sb.tile([P, QT], F32, tag="fsb", bufs=2,
                                      name="fsb")
                        nc.vector.tensor_copy(fsb, psf[i * 2 + t])
                        nc.sync.dma_start(
                            out_t[ds((g * 2 + i) * P, P), ds(t * QT, QT)],
                            fsb,
                        )

    nc.compile()
    return nc


# --------------------------------------------------------------------------
# Host-side prep and driver
# --------------------------------------------------------------------------

def _q_blocks(role, n_blocks):
    if n_blocks == 4:
        return [0, 3] if role == 0 else [1, 2]
    return list(range(n_blocks))


def make_host_data(x, w_in, w_out, q_gamma, q_beta, k_gamma, k_beta, cfg,
                   n_cores=None):
    import ml_dtypes

    BFNP = ml_dtypes.bfloat16

    D = cfg["D"]
    S = cfg["S"]
    NQTOK = cfg["NQTOK"]
    QT = cfg["QT"]
    masked = cfg["masked"]
    NH = D // HD
    DC = D // P
    NQ = NQTOK // QT
    MAXM = max(len(m) for m in masked)
    B = x.shape[0]
    n_blocks = S // 512
    if n_cores is None:
        n_cores = B * (2048 // NQTOK) if S == 2048 else B

    w64 = np.asarray(w_in, np.float64)
    wq = w64[0:D]
    wk = w64[D:2 * D]
    wv = w64[2 * D:3 * D]
    wq_c = wq - wq.mean(axis=0, keepdims=True)
    wk_c = wk - wk.mean(axis=0, keepdims=True)
    # wqk[ec, p, dc, e] = w_c.T[dc*P+p, ec*P+e]
    wqkT2 = np.concatenate([wq_c.T, wk_c.T], axis=1)  # [D, 2D]
    wqk = np.ascontiguousarray(
        wqkT2.reshape(DC, P, 2 * NH, P).transpose(2, 1, 0, 3)
    ).astype(BFNP)
    # wv[p, dc, e] = wv.T[dc*P+p, e]
    wv_t = np.ascontiguousarray(
        wv.T.reshape(DC, P, D).transpose(1, 0, 2)
    ).astype(BFNP)
    # wo[h, p, e] = w_out.T[h*P+p, e]
    wo_t = np.ascontiguousarray(
        np.asarray(w_out, np.float64).T.reshape(NH, P, D)
    ).astype(BFNP)

    inv = 1.0 / (10000.0 ** (np.arange(0, HD, 2, dtype=np.float64) / HD))
    tpos = np.arange(S, dtype=np.float64)
    fr = np.outer(tpos, inv)
    emb = np.concatenate([fr, fr], axis=-1)  # [S, HD]
    cosT = np.cos(emb).T  # [HD, S]
    sinT = np.sin(emb).T
    # rope via partition-swap: t2 = swap(ch * BN) with
    # BN[p] = -s(p)*sin (s = rotate_half sign), exploiting that the
    # frequency table is duplicated across halves (sin/cos half-symmetric)
    h2 = HD // 2
    sgn = np.ones((HD, 1), np.float64)
    sgn[h2:] = -1.0
    sinTs = sinT * sgn

    scale = 1.0 / math.sqrt(HD)
    gq_a = np.ascontiguousarray(
        (np.asarray(q_gamma, np.float64) * scale).reshape(NH, P).T
    ).astype(np.float32)
    bq_a = np.ascontiguousarray(
        (np.asarray(q_beta, np.float64) * scale).reshape(NH, P).T
    ).astype(np.float32)
    gk_a = np.ascontiguousarray(
        np.asarray(k_gamma, np.float32).reshape(NH, P).T
    )
    bk_a = np.ascontiguousarray(
        np.asarray(k_beta, np.float32).reshape(NH, P).T
    )

    kcos = np.ascontiguousarray(cosT).astype(BFNP)
    ksin = np.ascontiguousarray(sinTs).astype(BFNP)

    in_maps = []
    meta = []
    cores_per_batch = max(1, n_cores // B)
    xT_cache = {}
    for c in range(n_cores):
        b = c // cores_per_batch
        r = c % cores_per_batch
        blocks = _q_blocks(r if cores_per_batch > 1 else 0, n_blocks)
        blocks = blocks[: NQTOK // 512]
        qtok = np.concatenate(
            [np.arange(bk * 512, (bk + 1) * 512) for bk in blocks]
        )
        if b not in xT_cache:
            xb = np.asarray(x[b], np.float32)  # [S, D]
            xTb = np.ascontiguousarray(xb.T)   # [D, S]
            xT_cache[b] = (
                np.ascontiguousarray(xTb.reshape(DC, P, S)).astype(BFNP),
                xTb,
            )
        xT, xTb = xT_cache[b]
        # xq[p, dc, j] = x[qtok[j], dc*P+p]
        xq = np.ascontiguousarray(
            xTb[:, qtok].reshape(DC, P, NQTOK).transpose(1, 0, 2)
        ).astype(BFNP)
        qcos = np.ascontiguousarray(cosT[:, qtok]).astype(BFNP)
        qsin = np.ascontiguousarray(sinTs[:, qtok]).astype(BFNP)

        masks = np.zeros([NQ, P, MAXM, QT], np.float32)
        for t in range(NQ):
            assert QT == 512
            q_start = blocks[t] * 512
            qq = np.arange(QT)
            kk = np.arange(P)
            for mi, kc in enumerate(masked[t]):
                masks[t, :, mi, :] = (
                    (kc * P + kk[:, None]) <= (q_start + qq[None, :])
                ).astype(np.float32)
        masks = masks.astype(BFNP)

        in_maps.append(dict(
            xT=xT, xq=xq, wqk=wqk, wv=wv_t, wo=wo_t,
            qcos=qcos, qsin=qsin, kcos=kcos, ksin=ksin,
            gq=gq_a, bq=bq_a, gk=gk_a, bk=bk_a, masks=masks,
            onesc=np.ones((P, 1), BFNP),
            onesr=np.ones((1, P), BFNP),
            onescf=np.ones((P, 1), np.float32),
            onesrf=np.ones((1, P), np.float32),
        ))
        meta.append(dict(b=b, qtok=qtok))
    return in_maps, meta


_PROGRAM_CACHE = {}


def _get_program(cfg_key, cfg):
    if cfg_key not in _PROGRAM_CACHE:
        _PROGRAM_CACHE[cfg_key] = build_program(cfg)
    return _PROGRAM_CACHE[cfg_key]


def run_full(x, w_in, w_out, q_gamma, q_beta, k_gamma, k_beta,
             trace=False):
    from concourse.bass_utils import run_bass_kernel_spmd

    cfg = FULL_CFG
    B = x.shape[0]
    n_cores = 2 * B
    in_maps, meta = make_host_data(
        x, w_in, w_out, q_gamma, q_beta, k_gamma, k_beta, cfg,
        n_cores=n_cores,
    )
    nc = _get_program("full", cfg)
    res = run_bass_kernel_spmd(
        nc, in_maps, core_ids=list(range(n_cores)), trace=trace,
    )
    S, D = cfg["S"], cfg["D"]
    out = np.empty((B, S, D), np.float32)
    for c in range(n_cores):
        o = res.results[c]["out"]  # [D, NQTOK]
        out[meta[c]["b"], meta[c]["qtok"], :] = o.T
    return out, res


def kernel(x, w_in, w_out, q_gamma, q_beta, k_gamma, k_beta, n_heads=16,
           **_ignored):
    x = np.asarray(x, np.float32)
    assert int(np.asarray(n_heads)) * HD == x.shape[-1]
    out, _ = run_full(
        np.asarray(x, np.float32),
        np.asarray(w_in, np.float32),
        np.asarray(w_out, np.float32),
        np.asarray(q_gamma, np.float32),
        np.asarray(q_beta, np.float32),
        np.asarray(k_gamma, np.float32),
        np.asarray(k_beta, np.float32),
    )
    return out
